# revision 1
# baseline (speedup 1.0000x reference)
"""Trainium2 Bass kernel for the Cl(3,1) Clifford geometric product.

    out[b,t,c] = sum_{i,j} CAYLEY[i,j,c] * a[b,t,i] * b[b,t,j]

with a, b of shape (1024, 1024, 16) fp32.

Algorithm: Cl(3,1) is isomorphic to M4(R) (real Majorana gamma matrices).
Per position the geometric product becomes a 4x4 matrix product:

    out = Phi^-1 vec( mat(Phi a) @ mat(Phi b) )

The 16x16 transforms Phi / Phi^-1 are folded into block-diagonal 128x128
stationary matrices applied on the tensor engine (float32r, full rate),
with data held component-major ("transposed") so the 16-component axis
sits on SBUF partitions (8 position-groups x 16 components = 128).
The only per-position nonlinearity — the 64 products A[r,k]*B[k,m] —
runs on the vector engine as one elementwise multiply per slab.

Sharding: batch dim 1024 split 8 ways (128 rows / core); the host
re-lays each core's slice out component-major, the device streams 32
slabs of 512 columns, and the host inverts the layout on gather.

Raw-bass implementation (no TileContext): explicit per-engine programs
with standalone wait_ge instructions, because the self-loading float32r
matmul (and NoOp/Drain) instruction encodings only fit a single sync
wait, which the Tile scheduler's wait placement violates.
"""

import os
import sys

import numpy as np

for _p in ("/opt/trn_rl_repo", os.path.expanduser("~/.axon_site/_ro/trn_rl_repo")):
    if os.path.isdir(_p) and _p not in sys.path:
        sys.path.insert(0, _p)

N_CORES = 8
B_FULL = 1024  # batch rows
T_FULL = 1024  # positions per row
D = 16  # blade components
ROWS_PER_CORE = B_FULL // N_CORES  # 128
POS_PER_CORE = ROWS_PER_CORE * T_FULL  # 131072
G = 8  # position-groups stacked on partitions
COLS = POS_PER_CORE // G  # 16384 columns in the transposed layout
SLAB = 512  # columns per slab (one PSUM bank of fp32)
N_SLABS = COLS // SLAB  # 32


def _build_phi():
    """Phi[(4r+k), i] = rho(e_i)[r, k] for a real 4x4 rep of Cl(3,1)."""
    i2 = np.eye(2)
    sx = np.array([[0.0, 1.0], [1.0, 0.0]])
    sz = np.array([[1.0, 0.0], [0.0, -1.0]])
    ee = np.array([[0.0, 1.0], [-1.0, 0.0]])  # E^2 = -I
    gammas = [np.kron(sx, i2), np.kron(sz, i2), np.kron(ee, ee), np.kron(ee, sx)]
    phi = np.zeros((16, 16))
    for blade in range(16):
        mat = np.eye(4)
        for bit in range(4):
            if (blade >> bit) & 1:
                mat = mat @ gammas[bit]
        phi[:, blade] = mat.reshape(16)
    return phi, np.linalg.inv(phi)


def _build_weights():
    """The 9 block-diagonal 128x128 stationary matrices, stacked [128, 9*128].

    matmul(out, lhsT, rhs) computes out = lhsT.T @ rhs with the contraction
    over partitions, so each W is laid out [K, M] = [(g,j), (g,m)].
    """
    phi, phinv = _build_phi()
    mats = []

    # slot 0 unused (Phi is pre-applied to `a` on the host); keep the
    # [128, 9*128] layout so weight indices stay stable
    mats.append(np.zeros((128, 128), np.float32))

    # W_psi_m: Btil_m[(g,(r,k)), col] = Btil[(k,m)] = sum_j phi[(4k+m), j] b_j
    for m in range(4):
        blk = np.zeros((16, 16), np.float32)  # [j, (r,k)]
        for r in range(4):
            for k in range(4):
                blk[:, 4 * r + k] = phi[4 * k + m, :]
        w = np.zeros((128, 128), np.float32)
        for g in range(G):
            w[g * 16:(g + 1) * 16, g * 16:(g + 1) * 16] = blk
        mats.append(w)

    # W_theta_m: out[(g,c), col] += sum_{(r,k)} phinv[c, (4r+m)] * Z_m[(g,(r,k)), col]
    for m in range(4):
        blk = np.zeros((16, 16), np.float32)  # [(r,k), c]
        for r in range(4):
            for k in range(4):
                blk[4 * r + k, :] = phinv[:, 4 * r + m]
        w = np.zeros((128, 128), np.float32)
        for g in range(G):
            w[g * 16:(g + 1) * 16, g * 16:(g + 1) * 16] = blk
        mats.append(w)

    return np.concatenate(mats, axis=1)  # [128, 1152]


def _build_bass():
    import concourse.bass as bass
    import concourse.mybir as mybir

    f32 = mybir.dt.float32
    f32r = mybir.dt.float32r

    nc = bass.Bass(trn_type="TRN2")
    aT = nc.declare_dram_parameter("aT", [128, COLS], f32r, isOutput=False)
    bT = nc.declare_dram_parameter("bT", [128, COLS], f32r, isOutput=False)
    wm = nc.declare_dram_parameter("wm", [128, 9 * 128], f32r, isOutput=False)
    outT = nc.declare_dram_parameter("outT", [128, COLS], f32, isOutput=True)

    HB = SLAB // 2  # half-slab columns (256)

    # SBUF buffers (triple-buffered loads, double-buffered work tiles).
    # `aT` arrives with Phi pre-applied on the host, so at[] feeds the
    # vector multiply directly.
    wt = nc.alloc_sbuf_tensor("wt", [128, 9 * 128], f32r).ap()
    at = [nc.alloc_sbuf_tensor(f"at{i}", [128, SLAB], f32r).ap() for i in range(3)]
    bt = [nc.alloc_sbuf_tensor(f"bt{i}", [128, SLAB], f32r).ap() for i in range(3)]
    zt = [nc.alloc_sbuf_tensor(f"zt{i}", [128, 4 * SLAB], f32r).ap() for i in range(3)]
    ot = [nc.alloc_sbuf_tensor(f"ot{i}", [128, SLAB], f32).ap() for i in (0, 1)]

    # PSUM: pb 6 banks (3 rotating m-pair sets), po 2 banks
    pb_t = nc.alloc_psum_tensor("pb", [128, 6 * SLAB], f32)
    po_t = nc.alloc_psum_tensor("po", [128, 2 * SLAB], f32)
    po = [po_t.ap()[:, i * SLAB:(i + 1) * SLAB] for i in (0, 1)]
    pb = pb_t.ap()

    def w_ap(idx):
        return wt[:, idx * 128:(idx + 1) * 128]

    import contextlib

    with contextlib.ExitStack() as _st:
        block = _st.enter_context(nc.Block())
        sW = _st.enter_context(nc.semaphore("sW"))
        sW2 = _st.enter_context(nc.semaphore("sW2"))
        # one DMA-completion semaphore per buffer slot: DMAs issued to
        # different HW queues complete out of order, so a single cumulative
        # counter per stream would be racy
        sA = [_st.enter_context(nc.semaphore(f"sA{i}")) for i in range(3)]
        sB = [_st.enter_context(nc.semaphore(f"sB{i}")) for i in range(3)]
        sOd = [_st.enter_context(nc.semaphore(f"sOd{i}")) for i in range(2)]
        sPsiH = _st.enter_context(nc.semaphore("sPsiH"))
        sThetaH = _st.enter_context(nc.semaphore("sThetaH"))
        sDveH = _st.enter_context(nc.semaphore("sDveH"))
        sCo = _st.enter_context(nc.semaphore("sCo"))

        @block.sync
        def _(sync):
            # ramp: bt0 + Psi weights first so PE/DVE start ASAP
            sync.dma_start(out=bt[0], in_=bT[:, 0:SLAB]).then_inc(sB[0], 16)
            sync.dma_start(out=wt[:, 128:640], in_=wm[:, 128:640]).then_inc(sW, 16)
            sync.dma_start(out=at[0], in_=aT[:, 0:SLAB]).then_inc(sA[0], 16)
            sync.dma_start(out=wt[:, 640:1152], in_=wm[:, 640:1152]).then_inc(sW2, 16)
            for s in range(1, N_SLABS):
                i = s % 3
                if s >= 3:
                    # at[i]/bt[i] freed once DVE(s-3)/Psi(s-3) fully read them
                    sync.wait_ge(sDveH, 2 * (s - 3) + 2)
                    sync.wait_ge(sPsiH, 2 * (s - 3) + 2)
                c0 = s * SLAB
                sync.dma_start(out=at[i],
                               in_=aT[:, c0:c0 + SLAB]).then_inc(sA[i], 16)
                sync.dma_start(out=bt[i],
                               in_=bT[:, c0:c0 + SLAB]).then_inc(sB[i], 16)
            sync.wait_ge(sOd[0], 16 * (N_SLABS // 2))
            sync.wait_ge(sOd[1], 16 * (N_SLABS // 2))

        @block.tensor
        def _(pe):
            pe.wait_ge(sW, 16)

            def theta(t):
                # Theta MMs for slab t: consume zt[t%3] -> po[t%2]
                j = t % 2
                z_v = zt[t % 3].rearrange("p (col m) -> p m col", m=4)
                if t == 0:
                    pe.wait_ge(sW2, 16)  # Theta weights arrive last
                pe.wait_ge(sDveH, 2 * t + 2)
                if t >= 2:
                    pe.wait_ge(sCo, t - 1)  # po[j] freed by copy-o(t-2)
                for m in range(4):
                    mm = pe.matmul(out=po[j], lhsT=w_ap(5 + m),
                                   rhs=z_v[:, m:m + 1, :],
                                   start=(m == 0), stop=(m == 3))
                mm.then_inc(sThetaH)

            for s in range(N_SLABS):
                i = s % 3
                pe.wait_ge(sB[i], 16 * (s // 3 + 1))
                for half in (0, 1):  # m-bank pairs (0,1) then (2,3)
                    q = 2 * s + half
                    pair = q % 3
                    if q >= 3:
                        pe.wait_ge(sDveH, q - 2)  # pair freed 2 DVE-ops back
                    for j, m in enumerate((2 * half, 2 * half + 1)):
                        mm = pe.matmul(
                            out=pb[:, (2 * pair + j) * SLAB:
                                    (2 * pair + j + 1) * SLAB],
                            lhsT=w_ap(1 + m), rhs=bt[i],
                            start=True, stop=True)
                    mm.then_inc(sPsiH)
                if s >= 1:
                    theta(s - 1)
            theta(N_SLABS - 1)

        @block.vector
        def _(dve):
            for s in range(N_SLABS):
                i = s % 3
                z_o = zt[s % 3].rearrange("p (col m) -> p col m", m=4)
                for half in (0, 1):
                    q = 2 * s + half
                    pair = q % 3
                    pb_v = (pb[:, 2 * pair * SLAB:(2 * pair + 2) * SLAB]
                            .rearrange("p (m col) -> p col m", m=2))
                    dve.wait_ge(sPsiH, q + 1)
                    if half == 0:
                        dve.wait_ge(sA[i], 16 * (s // 3 + 1))
                        if s >= 3:
                            dve.wait_ge(sThetaH, s - 2)  # zt[s%3] freed
                    a_in = at[i].unsqueeze(2).broadcast_to((128, SLAB, 2))
                    dve.tensor_mul(z_o[:, :, 2 * half:2 * half + 2], a_in,
                                   pb_v).then_inc(sDveH)

        @block.scalar
        def _(act):
            for s in range(N_SLABS):
                j = s % 2
                act.wait_ge(sThetaH, s + 1)
                if s >= 2:
                    # ot[j] freed by out-dma(s-2), which ran on slot j
                    act.wait_ge(sOd[j], 16 * ((s - 2) // 2 + 1))
                act.copy(out=ot[j], in_=po[j]).then_inc(sCo)
                act.wait_ge(sCo, s + 1)  # DMA runs async: order after the copy
                c0 = s * SLAB
                act.dma_start(out=outT[:, c0:c0 + SLAB],
                              in_=ot[j]).then_inc(sOd[j], 16)

    return nc


_NC_CACHE = None


def _get_nc():
    global _NC_CACHE
    if _NC_CACHE is None:
        _NC_CACHE = _build_bass()
    return _NC_CACHE


def _to_transposed(x_core, pre=None):
    """(ROWS_PER_CORE, T_FULL, D) -> [128, COLS] component-major.

    If `pre` is given (16x16), applies x @ pre.T per position first.
    """
    flat = np.ascontiguousarray(x_core, dtype=np.float32).reshape(POS_PER_CORE, D)
    if pre is not None:
        flat = (flat @ pre.T.astype(np.float32))
    # row (g*16 + i), column t  <-  position (g*COLS + t), component i
    return np.ascontiguousarray(
        flat.reshape(G, COLS, D).transpose(0, 2, 1).reshape(128, COLS))


def _from_transposed(y):
    """[128, COLS] -> (POS_PER_CORE, D) natural layout."""
    return np.ascontiguousarray(
        y.reshape(G, D, COLS).transpose(0, 2, 1)).reshape(POS_PER_CORE, D)


def kernel(a, b):
    from concourse.bass_utils import run_bass_kernel_spmd

    a = np.asarray(a, dtype=np.float32)
    b = np.asarray(b, dtype=np.float32)
    assert a.shape == (B_FULL, T_FULL, D) and b.shape == a.shape

    wm = _build_weights()
    phi, _ = _build_phi()
    in_maps = []
    for c in range(N_CORES):
        sl = slice(c * ROWS_PER_CORE, (c + 1) * ROWS_PER_CORE)
        in_maps.append({
            "aT": _to_transposed(a[sl], pre=phi),
            "bT": _to_transposed(b[sl]),
            "wm": wm,
        })

    nc = _get_nc()
    res = run_bass_kernel_spmd(nc, in_maps, list(range(N_CORES)))

    out = np.empty((B_FULL, T_FULL, D), dtype=np.float32)
    for c in range(N_CORES):
        sl = slice(c * ROWS_PER_CORE, (c + 1) * ROWS_PER_CORE)
        out[sl] = _from_transposed(res.results[c]["outT"]).reshape(
            ROWS_PER_CORE, T_FULL, D)
    return out

